# revision 1
# baseline (speedup 1.0000x reference)
"""AffinityFC Trainium2 kernel (Bass/Tile, 8 NeuronCores, data-parallel over B).

Math per batch b (one NeuronCore per batch):
    px = X[b] @ W1x.T          (Nx=128, hd=1024)
    py = Y[b] @ W1y.T          (Ny=128, hd=1024)
    out[n, m] = W2 . relu(px[n, :] + py[m, :] + b1) + b2

Key reformulation: with s = px + b1,
    relu(py + s) = max(py, -s) + s
so the device computes u = max(py, -s) (a plain tensor_tensor max, no
broadcast materialization, no relu pass) and reduces Sum_h W2[h]*u with
TensorE; the Sum_h W2[h]*s[n,h] term is a rank-1 correction
gamma[b,n] = (W2@W1x)·X[b,n] + W2·b1 added on the host, along with b2.

Device layout (per core):
  - layer 1 (TensorE, bf16): per h-chunk c (8 chunks of 128):
      negs_c = -(px_c + b1_c)         (h, n) bf16   [DVE psum evac]
      py_rep4_c[hp, m*4+j] = py_c[hp, m]  (x4 interleaved) bf16 [ACT evac]
  - main loop, c-outer: for each chunk c, 4 octet TT ops produce
      t[hp, nbs*512 + m*4 + j] = max(py_c[hp,m], negs_c[hp, 4*nb+j])
    at 2x bf16 (all operands innermost step-1), then 32 matmuls with the
    same stationary W2_c column accumulate psum slivers
    (bank=nbs, partition=32*oct).  Sliver evacuation on ScalarE.
  - output raw layout: raw[nb*512 + m*4 + j] = out[4nb+j, m]; host
    unscrambles and adds gamma + b2.
"""

import numpy as np
import ml_dtypes

import concourse.mybir as mybir
import concourse.tile as tile
from concourse import bacc
from concourse.bass import ts
from concourse.bass_utils import run_bass_kernel_spmd

B, NX, NY, D, HD = 8, 128, 128, 512, 1024
NCORES = 8
NCH = HD // 128      # 8 h-chunks
KT = D // 128        # 4 k-tiles for the layer-1 contraction
NBLK = NX // 4       # 32 n-blocks of 4 rows each
OCTW = 8             # n-blocks per TT producer op (FD = OCTW*512)
NOCT = NBLK // OCTW  # 4 octets
F32 = mybir.dt.float32
BF16 = mybir.dt.bfloat16

# n-blocks produced on ScalarE (relu + per-partition bias) instead of DVE;
# the host zeroes gamma for these rows (they get true relu, no max-identity).
# They sit on banks 0,1 so the bank-major final pass isn't gated on ACT.
ACT_NBS = (24, 25)
DVE_RANGES = ((0, 8), (8, 16), (16, 24), (26, 32))


def _build_nc(do_compile=True):
    nc = bacc.Bacc(
        "TRN2", target_bir_lowering=False, debug=False, num_devices=NCORES
    )

    # all big inputs arrive as flat SBUF images: dram[p, col] == sbuf[p, col];
    # xt and yt share one image so the DMA moves 2KB per partition row
    xyt = nc.dram_tensor("xyt", [128, KT * (NX + NY)], BF16, kind="ExternalInput")
    w1xt = nc.dram_tensor("w1xt", [128, KT * HD], BF16, kind="ExternalInput")
    w1yt = nc.dram_tensor("w1yt", [128, KT * HD], BF16, kind="ExternalInput")
    b1c = nc.dram_tensor("b1c", [128, NCH], F32, kind="ExternalInput")
    # W2 chunk columns replicated x32 so the sliver matmuls run M=32 and
    # fill whole col-groups (evac then reads fully-written banks)
    w2c = nc.dram_tensor("w2c", [128, NCH * 32], BF16, kind="ExternalInput")
    out = nc.dram_tensor("out", [1, NBLK * 512], F32, kind="ExternalOutput")

    with tile.TileContext(nc) as tc:
        with (
            tc.tile_pool(name="const", bufs=1) as cp,
            tc.tile_pool(name="tprod", bufs=5) as tp,
        ):
            xyt_sb = cp.tile([128, KT * (NX + NY)], BF16)
            xt_sb = xyt_sb[:, : KT * NX]
            yt_sb = xyt_sb[:, KT * NX :]
            # W1 lives in separate tiles per chunk-group so layer-1 chunk c
            # only waits for its own DMA ({0}, {1}, {2,3}, {4..7})
            W1GRP = ((0, 1), (1, 2), (2, 4), (4, NCH))
            w1x_g = [
                cp.tile([128, (hi - lo) * KT * 128], BF16, name=f"w1x{lo}")
                for lo, hi in W1GRP
            ]
            w1y_g = [
                cp.tile([128, (hi - lo) * KT * 128], BF16, name=f"w1y{lo}")
                for lo, hi in W1GRP
            ]

            def w1slab(g, c, k):  # lhsT slab for (chunk c, k-tile)
                for (lo, hi), tile_ in zip(W1GRP, g):
                    if lo <= c < hi:
                        off = ((c - lo) * KT + k) * 128
                        return tile_[:, off : off + 128]
                raise AssertionError
            b1_sb = cp.tile([128, NCH], F32)
            w2_sb = cp.tile([128, NCH * 32], BF16)
            negs_sb = cp.tile([128, HD], BF16)
            s_sb = cp.tile([128, HD], F32)  # +s, f32 (ACT-path bias)
            pyr_sb = cp.tile([128, NCH * 512], BF16)  # py_rep4 per chunk
            # whole-bank evacuation scratch: bank b lands in columns b*512..;
            # only partitions 0/32/64/96 carry real slivers, the final DMA
            # gathers them with a partition-strided read
            out_sc = cp.tile([128, 8 * 512], F32)

            nc.sync.dma_start(out=xyt_sb[:, :], in_=xyt[:, :])
            nc.sync.dma_start(out=b1_sb[:, :], in_=b1c[:, :])
            nc.sync.dma_start(out=w2_sb[:, :], in_=w2c[:, :])
            # W1 as flat contiguous images; chunks 0 and 1 arrive as small
            # priority transfers so layer-1 (and the main loop) start early,
            # the rest streams behind on the same queues.
            CW = KT * 128  # image columns per chunk
            for w_g, w_dr, eng in (
                (w1x_g, w1xt, nc.scalar),
                (w1y_g, w1yt, nc.gpsimd),
            ):
                for (lo, hi), tile_ in zip(W1GRP, w_g):
                    eng.dma_start(
                        out=tile_[:, :],
                        in_=w_dr[:, lo * CW : hi * CW],
                    )

            # ---- layer 1 per h-chunk: negs (DVE) + py_rep4 (ACT)
            with tc.tile_pool(name="l1ps", bufs=4, space="PSUM") as l1ps:
                for c in range(NCH):
                    pxp = l1ps.tile([128, NX], F32, tag="l1")
                    for k in range(KT):
                        nc.tensor.matmul(
                            pxp[:, :],
                            w1slab(w1x_g, c, k),
                            xt_sb[:, ts(k, NX)],
                            start=(k == 0),
                            stop=(k == KT - 1),
                        )
                    nc.vector.tensor_scalar(
                        out=negs_sb[:, ts(c, 128)],
                        in0=pxp[:, :],
                        scalar1=b1_sb[:, c : c + 1],
                        scalar2=-1.0,
                        op0=mybir.AluOpType.add,
                        op1=mybir.AluOpType.mult,
                    )
                    nc.scalar.activation(
                        out=s_sb[:, ts(c, 128)],
                        in_=pxp[:, :],
                        func=mybir.ActivationFunctionType.Identity,
                        bias=b1_sb[:, c : c + 1],
                        scale=1.0,
                    )
                    pyp = l1ps.tile([128, NY], F32, tag="l1")
                    for k in range(KT):
                        nc.tensor.matmul(
                            pyp[:, :],
                            w1slab(w1y_g, c, k),
                            yt_sb[:, ts(k, NY)],
                            start=(k == 0),
                            stop=(k == KT - 1),
                        )
                    nc.scalar.activation(
                        out=pyr_sb[:, ts(c, 512)].rearrange(
                            "p (m j) -> p m j", j=4
                        ),
                        in_=pyp[:, :].unsqueeze(2).broadcast_to((128, 128, 4)),
                        func=mybir.ActivationFunctionType.Copy,
                    )

            # ---- main loop, c-outer: all 32 psum slivers stay resident and
            # accumulate across the 8 chunk passes; W2_c stays stationary
            # within a pass.  The last pass runs bank-major with evacuations
            # interleaved (a bank's 4 slivers evac while later banks matmul).
            with tc.tile_pool(name="mps", bufs=1, space="PSUM") as mps:
                obanks = [
                    mps.tile([128, 512], F32, name=f"ob{i}", tag=f"ob{i}")
                    for i in range(8)
                ]
                for c in range(NCH):
                    last = c == NCH - 1
                    pyr_c = pyr_sb[:, ts(c, 512)]
                    pyr3 = pyr_c.rearrange("p (m j) -> p m j", j=4)
                    # nb -> (tile, column offset) for this pass
                    tslice = {}
                    for gi, (lo, hi) in enumerate(DVE_RANGES):
                        w = hi - lo
                        t = tp.tile(
                            [128, w * 512], BF16, name=f"t{c}_{gi}", tag="t"
                        )
                        in0 = pyr3.unsqueeze(1).broadcast_to((128, w, 128, 4))
                        base = c * 128 + lo * 4
                        in1 = (
                            negs_sb[:, base : base + 4 * w]
                            .rearrange("p (nbs j) -> p nbs j", j=4)
                            .unsqueeze(2)
                            .broadcast_to((128, w, 128, 4))
                        )
                        if c == 0:
                            # schedule pass-0 producers ahead of later
                            # layer-1 evacs on the DVE queue
                            with tc.high_priority():
                                nc.vector.tensor_tensor(
                                    out=t[:, :].rearrange(
                                        "p (nbs m j) -> p nbs m j", nbs=w, m=128
                                    ),
                                    in0=in0,
                                    in1=in1,
                                    op=mybir.AluOpType.max,
                                )
                        else:
                            nc.vector.tensor_tensor(
                                out=t[:, :].rearrange(
                                    "p (nbs m j) -> p nbs m j", nbs=w, m=128
                                ),
                                in0=in0,
                                in1=in1,
                                op=mybir.AluOpType.max,
                            )
                        for nbs in range(w):
                            tslice[lo + nbs] = (t, nbs)
                    for nb in ACT_NBS:
                        ta = tp.tile(
                            [128, 512], BF16, name=f"ta{c}_{nb}", tag="ta", bufs=4
                        )
                        for j in range(4):
                            n = nb * 4 + j
                            nc.scalar.activation(
                                out=ta[:, :].rearrange("p (m j) -> p m j", j=4)[
                                    :, :, j
                                ],
                                in_=pyr3[:, :, j],
                                func=mybir.ActivationFunctionType.Relu,
                                bias=s_sb[:, c * 128 + n : c * 128 + n + 1],
                                scale=1.0,
                            )
                        tslice[nb] = (ta, 0)
                    if not last:
                        for nb in range(NBLK):
                            bk, jc = nb % 8, nb // 8
                            t, nbs = tslice[nb]
                            nc.tensor.matmul(
                                obanks[bk][32 * jc : 32 * jc + 32, :],
                                w2_sb[:, ts(c, 32)],
                                t[:, ts(nbs, 512)],
                                start=(c == 0),
                                stop=False,
                                tile_position=(0, 32 * jc),
                                skip_group_check=True,
                            )
                    else:
                        # bank-major: finish a bank's 4 slivers, evacuate them
                        # (DVE+ACT split) while the next bank matmuls
                        for bk in range(8):
                            for jc in range(NOCT):
                                nb = jc * 8 + bk
                                t, nbs = tslice[nb]
                                nc.tensor.matmul(
                                    obanks[bk][32 * jc : 32 * jc + 32, :],
                                    w2_sb[:, ts(c, 32)],
                                    t[:, ts(nbs, 512)],
                                    start=False,
                                    stop=True,
                                    tile_position=(0, 32 * jc),
                                    skip_group_check=True,
                                )
                            # one full-bank copy (128 lanes, same cost as one
                            # sliver); garbage rows are never DMA'd out
                            src = obanks[bk][:, :]
                            dst = out_sc[:, ts(bk, 512)]
                            if bk < 4:
                                nc.scalar.copy(out=dst, in_=src)
                            else:
                                nc.vector.tensor_copy(out=dst, in_=src)

            # raw[nb*512 + m*4 + j] with nb = jc*8 + bk: partition-strided
            # gather of rows 0/32/64/96 from the bank-evac scratch
            nc.sync.dma_start(
                out=out[:, :].rearrange(
                    "o (jc bk q) -> (o jc) bk q", jc=4, bk=8
                ),
                in_=out_sc[0:128:32, :].rearrange("jc (bk q) -> jc bk q", bk=8),
            )

    if do_compile:
        nc.compile()
    return nc


_NC_CACHE = None


def _get_nc():
    global _NC_CACHE
    if _NC_CACHE is None:
        _NC_CACHE = _build_nc()
    return _NC_CACHE


def prepare_in_maps(X, Y, W1, b1, W2):
    X = np.asarray(X, dtype=np.float32)
    Y = np.asarray(Y, dtype=np.float32)
    W1 = np.asarray(W1, dtype=np.float32)
    b1 = np.asarray(b1, dtype=np.float32)
    W2 = np.asarray(W2, dtype=np.float32)

    bf = ml_dtypes.bfloat16

    def w1_img(Wh):  # (HD, D) -> flat sbuf image (128, KT*HD)
        return np.ascontiguousarray(
            Wh.reshape(NCH, 128, KT, 128).transpose(3, 0, 2, 1).reshape(128, -1)
        ).astype(bf)

    def xy_img(Xb):  # (N, D) -> flat sbuf image (128, KT*N)
        return np.ascontiguousarray(
            Xb.T.reshape(KT, 128, -1).transpose(1, 0, 2).reshape(128, -1)
        ).astype(bf)

    w1xt = w1_img(W1[:, :D])
    w1yt = w1_img(W1[:, D:])
    b1cm = np.ascontiguousarray(b1.reshape(NCH, 128).T)       # (128, NCH) f32
    w2cm = np.ascontiguousarray(
        np.repeat(W2.reshape(NCH, 128).T[:, :, None], 32, axis=2).reshape(128, -1)
    ).astype(bf)

    in_maps = []
    for b in range(B):
        in_maps.append(
            {
                "xyt": np.ascontiguousarray(
                    np.hstack([xy_img(X[b]), xy_img(Y[b])])
                ),
                "w1xt": w1xt,
                "w1yt": w1yt,
                "b1c": b1cm,
                "w2c": w2cm,
            }
        )
    return in_maps


def postprocess(raw_outs, X, W1, b1, W2, b2):
    """raw[nb*512 + m*4 + j] = Sum_h W2[h]*u for out row 4nb+j, col m.
    Add gamma[b,n] = (W2@W1x)·X[b,n] + W2·b1, then b2."""
    X = np.asarray(X, dtype=np.float32)
    W1 = np.asarray(W1, dtype=np.float32)
    b1 = np.asarray(b1, dtype=np.float32)
    W2 = np.asarray(W2, dtype=np.float32)
    b2 = np.asarray(b2, dtype=np.float32)

    v = W2[0] @ W1[:, :D]                     # (D,)
    gconst = float(W2[0] @ b1)
    out = np.empty((B, NX, NY), dtype=np.float32)
    for b in range(B):
        r = raw_outs[b].reshape(NBLK, 128, 4)     # (nb, m, j)
        o = r.transpose(0, 2, 1).reshape(NX, NY)  # (4nb+j, m)
        gamma = X[b] @ v + gconst                 # (NX,)
        for nb in ACT_NBS:                        # ACT rows got true relu
            gamma[nb * 4 : (nb + 1) * 4] = 0.0
        out[b] = o + gamma[:, None] + b2[0]
    return out


def kernel(X, Y, W1, b1, W2, b2):
    in_maps = prepare_in_maps(X, Y, W1, b1, W2)
    nc = _get_nc()
    res = run_bass_kernel_spmd(nc, in_maps, core_ids=list(range(NCORES)))
    raw = [res.results[b]["out"].reshape(-1) for b in range(B)]
    return postprocess(raw, X, W1, b1, W2, b2)


if __name__ == "__main__":
    rng = np.random.default_rng(0)
    ins = {
        "X": rng.standard_normal((B, NX, D), dtype=np.float32),
        "Y": rng.standard_normal((B, NY, D), dtype=np.float32),
        "W1": rng.standard_normal((HD, 2 * D), dtype=np.float32) * (2 * D) ** -0.5,
        "b1": rng.standard_normal((HD,), dtype=np.float32) * (2 * D) ** -0.5,
        "W2": rng.standard_normal((1, HD), dtype=np.float32) * HD**-0.5,
        "b2": rng.standard_normal((1,), dtype=np.float32) * HD**-0.5,
    }
    o = kernel(**ins)
    print("kernel out:", o.shape, o.dtype, float(np.abs(o).max()))



# revision 11
# speedup vs baseline: 1.1757x; 1.1757x over previous
"""AffinityFC Trainium2 kernel (Bass/Tile, 8 NeuronCores, data-parallel over B).

Math per batch b (one NeuronCore per batch):
    px = X[b] @ W1x.T          (Nx=128, hd=1024)
    py = Y[b] @ W1y.T          (Ny=128, hd=1024)
    out[n, m] = W2 . relu(px[n, :] + py[m, :] + b1) + b2

Key reformulation: with s = px + b1,
    relu(py + s) = max(py, -s) + s
so for "max-form" rows the device computes u = max(py, -s) (a plain
tensor_tensor max) and reduces Sum_h W2[h]*u with TensorE; the missing
Sum_h W2[h]*s[n,h] term is a per-chunk rank-1 correction added on the
host.  "relu-form" rows (produced on ScalarE via relu(py + s_n) with a
per-partition bias) need no correction for those chunks.

v2 layout/schedule (vs baseline):
  - work split per h-chunk c: chunks 1..6: DVE nb 0..23 (3 ops w=8),
    GPSIMD nb 24..28 (1 op w=5), ACT nb 29..31 (relu-form); chunk 0:
    all-DVE; chunk 7: all-DVE in 8 bank-grouped ops (w=4, nb stride 8)
    so the final bank-major matmul+evac pipeline starts per-bank.
  - layer-1 psum pool bufs=2 (2 banks) so the 8 output banks free up
    early; t-producer pool bufs=8.
  - input DMAs split into ~128-256KB pieces spread over all 5 engine
    queues, chunk-0/1-critical pieces first in each queue's FIFO.
  - final pass: ScalarE-only bank evacs + per-bank output DMAs.
"""

import numpy as np
import ml_dtypes

import concourse.mybir as mybir
import concourse.tile as tile
from concourse import bacc
from concourse.bass import ts
from concourse.bass_utils import run_bass_kernel_spmd

B, NX, NY, D, HD = 8, 128, 128, 512, 1024
NCORES = 8
NCH = HD // 128      # 8 h-chunks
KT = D // 128        # 4 k-tiles for the layer-1 contraction
NBLK = NX // 4       # 32 n-blocks of 4 rows each
F32 = mybir.dt.float32
BF16 = mybir.dt.bfloat16

# per-chunk producer split for the middle chunks (1..NCH-2):
DVE_RANGES = ((0, 8), (8, 16), (16, 24), (24, 28))  # nb on DVE (max-form)
ACT_NBS = (28, 29, 30, 31)                 # nb on ScalarE (relu-form)
ALLDVE_CHUNKS = (0, NCH - 1)               # chunks where every nb is DVE max-form


def _build_nc(do_compile=True):
    nc = bacc.Bacc(
        "TRN2",
        target_bir_lowering=False,
        debug=False,
        num_devices=NCORES,
        num_swdge_queues=4,
    )

    # flat SBUF images: dram[p, col] == sbuf[p, col]
    xt = nc.dram_tensor("xt", [128, KT * NX], BF16, kind="ExternalInput")
    yt = nc.dram_tensor("yt", [128, KT * NY], BF16, kind="ExternalInput")
    w1xt = nc.dram_tensor("w1xt", [128, KT * HD], BF16, kind="ExternalInput")
    w1yt = nc.dram_tensor("w1yt", [128, KT * HD], BF16, kind="ExternalInput")
    b1c = nc.dram_tensor("b1c", [128, NCH], F32, kind="ExternalInput")
    # W2 chunk columns replicated x32 so the sliver matmuls run M=32
    w2c = nc.dram_tensor("w2c", [128, NCH * 32], BF16, kind="ExternalInput")
    out = nc.dram_tensor("out", [1, NBLK * 512], F32, kind="ExternalOutput")

    with tile.TileContext(nc) as tc:
        with (
            tc.tile_pool(name="const", bufs=1) as cp,
            tc.tile_pool(name="tprod", bufs=8) as tp,
        ):
            xt_sb = cp.tile([128, KT * NX], BF16)
            yt_sb = cp.tile([128, KT * NY], BF16)
            # W1 tiles per chunk-group: c0 and c1 get their own small tiles
            # (arrive first), the rest stream behind in pairs.
            W1GRP = ((0, 1), (1, 2), (2, 4), (4, 6), (6, NCH))
            w1x_g = [
                cp.tile([128, (hi - lo) * KT * 128], BF16, name=f"w1x{lo}")
                for lo, hi in W1GRP
            ]
            w1y_g = [
                cp.tile([128, (hi - lo) * KT * 128], BF16, name=f"w1y{lo}")
                for lo, hi in W1GRP
            ]

            def w1slab(g, c, k):  # lhsT slab for (chunk c, k-tile)
                for (lo, hi), tile_ in zip(W1GRP, g):
                    if lo <= c < hi:
                        off = ((c - lo) * KT + k) * 128
                        return tile_[:, off : off + 128]
                raise AssertionError
            b1_sb = cp.tile([128, NCH], F32)
            w2_sb = cp.tile([128, NCH * 32], BF16)
            negs_sb = cp.tile([128, HD], BF16)
            s_sb = cp.tile([128, HD], F32)  # +s, f32 (ACT-path bias)
            pyr_sb = cp.tile([128, NCH * 512], BF16)  # py_rep4 per chunk
            out_sc = cp.tile([128, 8 * 512], F32)

            CW = KT * 128  # image columns per chunk
            HX = KT * NX // 2

            # critical-first FIFO order per queue: chunk-0/1 pieces lead,
            # bulk W1 streams behind on the same 3 queues
            nc.sync.dma_start(out=b1_sb[:, :], in_=b1c[:, :])
            nc.sync.dma_start(out=w2_sb[:, :], in_=w2c[:, :])
            nc.sync.dma_start(out=xt_sb[:, :HX], in_=xt[:, :HX])
            nc.scalar.dma_start(out=w1x_g[0][:, :], in_=w1xt[:, 0:CW])
            nc.gpsimd.dma_start(out=w1y_g[0][:, :], in_=w1yt[:, 0:CW])
            nc.sync.dma_start(out=xt_sb[:, HX:], in_=xt[:, HX:])
            nc.scalar.dma_start(out=yt_sb[:, :HX], in_=yt[:, :HX])
            nc.gpsimd.dma_start(out=yt_sb[:, HX:], in_=yt[:, HX:])
            nc.sync.dma_start(out=w1x_g[1][:, :], in_=w1xt[:, CW : 2 * CW])
            nc.gpsimd.dma_start(out=w1y_g[1][:, :], in_=w1yt[:, CW : 2 * CW])
            # bulk
            nc.scalar.dma_start(out=w1x_g[2][:, :], in_=w1xt[:, 2 * CW : 4 * CW])
            nc.gpsimd.dma_start(out=w1y_g[2][:, :], in_=w1yt[:, 2 * CW : 4 * CW])
            nc.sync.dma_start(out=w1x_g[3][:, :], in_=w1xt[:, 4 * CW : 6 * CW])
            nc.gpsimd.dma_start(out=w1y_g[3][:, :], in_=w1yt[:, 4 * CW : 6 * CW])
            nc.scalar.dma_start(out=w1x_g[4][:, :], in_=w1xt[:, 6 * CW : 8 * CW])
            nc.sync.dma_start(out=w1y_g[4][:, :], in_=w1yt[:, 6 * CW : 8 * CW])

            # ---- layer 1 per h-chunk: negs (DVE) + s (ACT) + py_rep4 (ACT)
            with tc.tile_pool(name="l1ps", bufs=2, space="PSUM") as l1ps:
                for c in range(NCH):
                    pxp = l1ps.tile([128, NX], F32, tag="l1")
                    for k in range(KT):
                        nc.tensor.matmul(
                            pxp[:, :],
                            w1slab(w1x_g, c, k),
                            xt_sb[:, ts(k, NX)],
                            start=(k == 0),
                            stop=(k == KT - 1),
                        )
                    nc.vector.tensor_scalar(
                        out=negs_sb[:, ts(c, 128)],
                        in0=pxp[:, :],
                        scalar1=b1_sb[:, c : c + 1],
                        scalar2=-1.0,
                        op0=mybir.AluOpType.add,
                        op1=mybir.AluOpType.mult,
                    )
                    nc.scalar.activation(
                        out=s_sb[:, ts(c, 128)],
                        in_=pxp[:, :],
                        func=mybir.ActivationFunctionType.Identity,
                        bias=b1_sb[:, c : c + 1],
                        scale=1.0,
                    )
                    pyp = l1ps.tile([128, NY], F32, tag="l1")
                    for k in range(KT):
                        nc.tensor.matmul(
                            pyp[:, :],
                            w1slab(w1y_g, c, k),
                            yt_sb[:, ts(k, NY)],
                            start=(k == 0),
                            stop=(k == KT - 1),
                        )
                    nc.scalar.activation(
                        out=pyr_sb[:, ts(c, 512)].rearrange(
                            "p (m j) -> p m j", j=4
                        ),
                        in_=pyp[:, :].unsqueeze(2).broadcast_to((128, 128, 4)),
                        func=mybir.ActivationFunctionType.Copy,
                    )

            # ---- main loop, c-outer: all 32 psum slivers stay resident and
            # accumulate across the 8 chunk passes.
            with tc.tile_pool(name="mps", bufs=1, space="PSUM") as mps:
                obanks = [
                    mps.tile([128, 512], F32, name=f"ob{i}", tag=f"ob{i}")
                    for i in range(8)
                ]
                for c in range(NCH):
                    last = c == NCH - 1
                    pyr_c = pyr_sb[:, ts(c, 512)]
                    pyr3 = pyr_c.rearrange("p (m j) -> p m j", j=4)
                    tslice = {}  # nb -> (tile, column offset index)

                    def dve_max(t, w, in1, prio):
                        in0 = pyr3.unsqueeze(1).broadcast_to((128, w, 128, 4))
                        out_ap = t[:, :].rearrange(
                            "p (nbs m j) -> p nbs m j", nbs=w, m=128
                        )
                        if prio:
                            with tc.high_priority():
                                nc.vector.tensor_tensor(
                                    out=out_ap, in0=in0, in1=in1,
                                    op=mybir.AluOpType.max,
                                )
                        else:
                            nc.vector.tensor_tensor(
                                out=out_ap, in0=in0, in1=in1,
                                op=mybir.AluOpType.max,
                            )

                    if last:
                        # bank-grouped: op bk covers nb {bk, 8+bk, 16+bk, 24+bk}
                        # so the bank-major final pass pipelines per-bank
                        for bk in range(8):
                            t = tp.tile([128, 4 * 512], BF16, name=f"tb{bk}", tag="t")
                            in1 = (
                                negs_sb[:, ts(c, 128)]
                                .rearrange("p (kk r) -> p kk r", kk=4)
                                [:, :, 4 * bk : 4 * bk + 4]
                                .unsqueeze(2)
                                .broadcast_to((128, 4, 128, 4))
                            )
                            dve_max(t, 4, in1, False)
                            for kk in range(4):
                                tslice[kk * 8 + bk] = (t, kk)
                    elif c in ALLDVE_CHUNKS:
                        for gi in range(4):
                            lo = gi * 8
                            t = tp.tile([128, 8 * 512], BF16, name=f"t{c}_{gi}", tag="t")
                            base = c * 128 + lo * 4
                            in1 = (
                                negs_sb[:, base : base + 32]
                                .rearrange("p (nbs j) -> p nbs j", j=4)
                                .unsqueeze(2)
                                .broadcast_to((128, 8, 128, 4))
                            )
                            dve_max(t, 8, in1, c == 0)
                            for nbs in range(8):
                                tslice[lo + nbs] = (t, nbs)
                    else:
                        for gi, (lo, hi) in enumerate(DVE_RANGES):
                            w = hi - lo
                            t = tp.tile([128, w * 512], BF16, name=f"t{c}_{gi}", tag="t")
                            base = c * 128 + lo * 4
                            in1 = (
                                negs_sb[:, base : base + 4 * w]
                                .rearrange("p (nbs j) -> p nbs j", j=4)
                                .unsqueeze(2)
                                .broadcast_to((128, w, 128, 4))
                            )
                            dve_max(t, w, in1, False)
                            for nbs in range(w):
                                tslice[lo + nbs] = (t, nbs)
                        # ACT share (relu-form rows)
                        for nb in ACT_NBS:
                            ta = tp.tile(
                                [128, 512], BF16, name=f"ta{c}_{nb}", tag="ta", bufs=4
                            )
                            for j in range(4):
                                n = nb * 4 + j
                                nc.scalar.activation(
                                    out=ta[:, :].rearrange("p (m j) -> p m j", j=4)[
                                        :, :, j
                                    ],
                                    in_=pyr3[:, :, j],
                                    func=mybir.ActivationFunctionType.Relu,
                                    bias=s_sb[:, c * 128 + n : c * 128 + n + 1],
                                    scale=1.0,
                                )
                            tslice[nb] = (ta, 0)

                    if not last:
                        for nb in range(NBLK):
                            bk, jc = nb % 8, nb // 8
                            t, nbs = tslice[nb]
                            nc.tensor.matmul(
                                obanks[bk][32 * jc : 32 * jc + 32, :],
                                w2_sb[:, ts(c, 32)],
                                t[:, ts(nbs, 512)],
                                start=(c == 0),
                                stop=False,
                                tile_position=(0, 32 * jc),
                                skip_group_check=True,
                            )
                    else:
                        # bank-major: finish a bank's 4 slivers, evacuate on
                        # ScalarE, DMA that bank's rows out — all pipelined
                        for bk in range(8):
                            for jc in range(4):
                                nb = jc * 8 + bk
                                t, nbs = tslice[nb]
                                nc.tensor.matmul(
                                    obanks[bk][32 * jc : 32 * jc + 32, :],
                                    w2_sb[:, ts(c, 32)],
                                    t[:, ts(nbs, 512)],
                                    start=False,
                                    stop=True,
                                    tile_position=(0, 32 * jc),
                                    skip_group_check=True,
                                )
                            nc.scalar.copy(
                                out=out_sc[:, ts(bk, 512)], in_=obanks[bk][:, :]
                            )
                            # raw[nb*512 + m*4 + j], nb = jc*8 + bk: gather
                            # rows 0/32/64/96 of this bank's evac scratch
                            dst = out[:, :].rearrange(
                                "o (jc bk q) -> (o jc) bk q", jc=4, bk=8
                            )[:, bk, :]
                            src = out_sc[0:128:32, ts(bk, 512)]
                            (nc.sync, nc.gpsimd, nc.scalar)[bk % 3].dma_start(
                                out=dst, in_=src
                            )

    if do_compile:
        nc.compile()
    return nc


_NC_CACHE = None


def _get_nc():
    global _NC_CACHE
    if _NC_CACHE is None:
        _NC_CACHE = _build_nc()
    return _NC_CACHE


def prepare_in_maps(X, Y, W1, b1, W2):
    X = np.asarray(X, dtype=np.float32)
    Y = np.asarray(Y, dtype=np.float32)
    W1 = np.asarray(W1, dtype=np.float32)
    b1 = np.asarray(b1, dtype=np.float32)
    W2 = np.asarray(W2, dtype=np.float32)

    bf = ml_dtypes.bfloat16

    def w1_img(Wh):  # (HD, D) -> flat sbuf image (128, KT*HD)
        return np.ascontiguousarray(
            Wh.reshape(NCH, 128, KT, 128).transpose(3, 0, 2, 1).reshape(128, -1)
        ).astype(bf)

    def xy_img(Xb):  # (N, D) -> flat sbuf image (128, KT*N)
        return np.ascontiguousarray(
            Xb.T.reshape(KT, 128, -1).transpose(1, 0, 2).reshape(128, -1)
        ).astype(bf)

    w1xt = w1_img(W1[:, :D])
    w1yt = w1_img(W1[:, D:])
    b1cm = np.ascontiguousarray(b1.reshape(NCH, 128).T)       # (128, NCH) f32
    w2cm = np.ascontiguousarray(
        np.repeat(W2.reshape(NCH, 128).T[:, :, None], 32, axis=2).reshape(128, -1)
    ).astype(bf)

    in_maps = []
    for b in range(B):
        in_maps.append(
            {
                "xt": xy_img(X[b]),
                "yt": xy_img(Y[b]),
                "w1xt": w1xt,
                "w1yt": w1yt,
                "b1c": b1cm,
                "w2c": w2cm,
            }
        )
    return in_maps


def postprocess(raw_outs, X, W1, b1, W2, b2):
    """raw[nb*512 + m*4 + j] = device sum for out row 4nb+j, col m.
    Add the per-(row, chunk-set) max-form correction gamma, then b2."""
    X = np.asarray(X, dtype=np.float32)
    W1 = np.asarray(W1, dtype=np.float32)
    b1 = np.asarray(b1, dtype=np.float32)
    W2 = np.asarray(W2, dtype=np.float32)
    b2 = np.asarray(b2, dtype=np.float32)

    # per-chunk rank-1 pieces: gam_c[b, n] = X[b, n]·v_c + g_c
    # v_c = W2_c @ W1x_c-slice ; g_c = W2_c·b1_c
    Vc = np.stack(
        [W2[0, ts_] @ W1[ts_, :D] for ts_ in (slice(c * 128, (c + 1) * 128) for c in range(NCH))]
    )  # (NCH, D)
    gc = np.array([W2[0, c * 128 : (c + 1) * 128] @ b1[c * 128 : (c + 1) * 128] for c in range(NCH)])
    # maxform chunk mask per nb: ACT rows are relu-form in middle chunks
    maxform = np.ones((NBLK, NCH), dtype=np.float32)
    for nb in ACT_NBS:
        for c in range(NCH):
            if c not in ALLDVE_CHUNKS:
                maxform[nb, c] = 0.0

    out = np.empty((B, NX, NY), dtype=np.float32)
    for b in range(B):
        r = raw_outs[b].reshape(NBLK, 128, 4)     # (nb, m, j)
        o = r.transpose(0, 2, 1).reshape(NX, NY)  # (4nb+j, m)
        A = X[b] @ Vc.T + gc                      # (NX, NCH)
        gamma = (A.reshape(NBLK, 4, NCH) * maxform[:, None, :]).sum(-1)
        out[b] = o + gamma.reshape(NX)[:, None] + b2[0]
    return out


def kernel(X, Y, W1, b1, W2, b2):
    in_maps = prepare_in_maps(X, Y, W1, b1, W2)
    nc = _get_nc()
    res = run_bass_kernel_spmd(nc, in_maps, core_ids=list(range(NCORES)))
    raw = [res.results[b]["out"].reshape(-1) for b in range(B)]
    return postprocess(raw, X, W1, b1, W2, b2)


if __name__ == "__main__":
    rng = np.random.default_rng(0)
    ins = {
        "X": rng.standard_normal((B, NX, D), dtype=np.float32),
        "Y": rng.standard_normal((B, NY, D), dtype=np.float32),
        "W1": rng.standard_normal((HD, 2 * D), dtype=np.float32) * (2 * D) ** -0.5,
        "b1": rng.standard_normal((HD,), dtype=np.float32) * (2 * D) ** -0.5,
        "W2": rng.standard_normal((1, HD), dtype=np.float32) * HD**-0.5,
        "b2": rng.standard_normal((1,), dtype=np.float32) * HD**-0.5,
    }
    o = kernel(**ins)
    print("kernel out:", o.shape, o.dtype, float(np.abs(o).max()))


# revision 17
# speedup vs baseline: 1.3286x; 1.1301x over previous
"""AffinityFC Trainium2 kernel (Bass/Tile, 8 NeuronCores, data-parallel over B).

Math per batch b (one NeuronCore per batch):
    px = X[b] @ W1x.T          (Nx=128, hd=1024)
    py = Y[b] @ W1y.T          (Ny=128, hd=1024)
    out[n, m] = W2 . relu(px[n, :] + py[m, :] + b1) + b2

Key reformulation: with s = px + b1,
    relu(py + s) = max(py, -s) + s
so for "max-form" rows the device computes u = max(py, -s) (one DVE
tensor_tensor max per element) and reduces Sum_h W2[h]*u with TensorE;
the missing Sum_h W2[h]*s[n,h] term is a per-chunk rank-1 correction
added on the host.  "relu-form" rows (ScalarE relu(py + s_n), bias per
partition) need no correction for their chunks.

v3 schedule:
  - PSUM: obanks 0..5 allocated BEFORE the layer-1 pool (disjoint stack
    space, so main matmuls for those banks start immediately); obanks
    6,7 allocated after layer-1's pool closes and reuse its space.
    Bank map: nb 0..23 -> bank nb%6 (w8 DVE tiles), nb 24..27 -> bank 7
    (w4 DVE tile), nb 28..31 -> bank 6 (ACT relu-form rows).
  - DVE does ONLY the max ops; negs/s/pyr layer-1 evacs all run on
    ScalarE at high priority so layer-1 never waits on the DVE.
  - input DMAs: critical chunk-0/1 pieces on the two HWDGE queues
    (sync, scalar), all bulk W1 on the gpsimd SWDGE queue.
  - last chunk: 8 bank-grouped w4 DVE ops feeding a per-bank
    matmul -> ScalarE evac -> small out-DMA pipeline.
"""

import numpy as np
import ml_dtypes

import concourse.mybir as mybir
import concourse.tile as tile
from concourse import bacc
from concourse.bass import ts
from concourse.bass_utils import run_bass_kernel_spmd

B, NX, NY, D, HD = 8, 128, 128, 512, 1024
NCORES = 8
NCH = HD // 128      # 8 h-chunks
KT = D // 128        # 4 k-tiles for the layer-1 contraction
NBLK = NX // 4       # 32 n-blocks of 4 rows each
F32 = mybir.dt.float32
BF16 = mybir.dt.bfloat16

ACT_NBS = (28, 29, 30, 31)   # relu-form rows (ScalarE) in middle chunks
ALLDVE_CHUNKS = (0, NCH - 1)


def bankmap(nb):  # nb -> (bank, jc)
    if nb < 24:
        return nb % 6, nb // 6
    if nb < 28:
        return 7, nb - 24
    return 6, nb - 28


def _build_nc(do_compile=True):
    nc = bacc.Bacc(
        "TRN2", target_bir_lowering=False, debug=False, num_devices=NCORES
    )

    # flat SBUF images: dram[p, col] == sbuf[p, col]
    xt = nc.dram_tensor("xt", [128, KT * NX], BF16, kind="ExternalInput")
    yt = nc.dram_tensor("yt", [128, KT * NY], BF16, kind="ExternalInput")
    w1xt = nc.dram_tensor("w1xt", [128, KT * HD], BF16, kind="ExternalInput")
    w1yt = nc.dram_tensor("w1yt", [128, KT * HD], BF16, kind="ExternalInput")
    b1c = nc.dram_tensor("b1c", [128, 2 * NCH], F32, kind="ExternalInput")
    w2c = nc.dram_tensor("w2c", [128, NCH * 32], BF16, kind="ExternalInput")
    out = nc.dram_tensor("out", [1, NBLK * 512], F32, kind="ExternalOutput")

    with tile.TileContext(nc) as tc:
        with (
            tc.tile_pool(name="const", bufs=1) as cp,
            tc.tile_pool(name="tprod", bufs=9) as tp,
        ):
            xt_sb = cp.tile([128, KT * NX], BF16)
            yt_sb = cp.tile([128, KT * NY], BF16)
            W1GRP = ((0, 1), (1, 2), (2, 4), (4, 6), (6, NCH))
            w1x_g = [
                cp.tile([128, (hi - lo) * KT * 128], BF16, name=f"w1x{lo}")
                for lo, hi in W1GRP
            ]
            w1y_g = [
                cp.tile([128, (hi - lo) * KT * 128], BF16, name=f"w1y{lo}")
                for lo, hi in W1GRP
            ]

            def w1slab(g, c, k):  # lhsT slab for (chunk c, k-tile)
                for (lo, hi), tile_ in zip(W1GRP, g):
                    if lo <= c < hi:
                        off = ((c - lo) * KT + k) * 128
                        return tile_[:, off : off + 128]
                raise AssertionError
            b1_sb = cp.tile([128, 2 * NCH], F32)    # [+b1 | -b1] chunk columns
            w2_sb = cp.tile([128, NCH * 32], BF16)
            negs_sb = cp.tile([128, HD], BF16)
            s_sb = cp.tile([128, HD], F32)
            pyr_sb = cp.tile([128, NCH * 512], BF16)  # py_rep4 per chunk
            out_sc = cp.tile([128, 8 * 512], F32)

            CW = KT * 128  # image columns per chunk
            # critical pieces on the 2 HWDGE queues, bulk W1 on SWDGE
            nc.sync.dma_start(out=xt_sb[:, :], in_=xt[:, :])
            nc.scalar.dma_start(out=w1x_g[0][:, :], in_=w1xt[:, 0:CW])
            nc.gpsimd.dma_start(out=w1y_g[0][:, :], in_=w1yt[:, 0:CW])
            nc.sync.dma_start(out=b1_sb[:, :], in_=b1c[:, :])
            nc.sync.dma_start(out=w2_sb[:, :], in_=w2c[:, :])
            nc.scalar.dma_start(out=yt_sb[:, :], in_=yt[:, :])
            nc.sync.dma_start(out=w1x_g[1][:, :], in_=w1xt[:, CW : 2 * CW])
            nc.scalar.dma_start(out=w1y_g[1][:, :], in_=w1yt[:, CW : 2 * CW])
            # bulk on gpsimd SWDGE only, in chunk order
            nc.gpsimd.dma_start(out=w1x_g[2][:, :], in_=w1xt[:, 2 * CW : 4 * CW])
            nc.gpsimd.dma_start(out=w1y_g[2][:, :], in_=w1yt[:, 2 * CW : 4 * CW])
            nc.gpsimd.dma_start(out=w1x_g[3][:, :], in_=w1xt[:, 4 * CW : 6 * CW])
            nc.gpsimd.dma_start(out=w1y_g[3][:, :], in_=w1yt[:, 4 * CW : 6 * CW])
            nc.gpsimd.dma_start(out=w1x_g[4][:, :], in_=w1xt[:, 6 * CW : 8 * CW])
            nc.gpsimd.dma_start(out=w1y_g[4][:, :], in_=w1yt[:, 6 * CW : 8 * CW])

            with tc.tile_pool(name="mpsA", bufs=1, space="PSUM") as mpsA:
                obanks = [None] * 8
                for i in range(6):
                    obanks[i] = mpsA.tile([128, 512], F32, name=f"ob{i}", tag=f"ob{i}")

                # ---- layer 1 per h-chunk; all evacs on ScalarE (hi-pri):
                #   negs = -(px+b1) bf16, s = px+b1 f32, pyr = py rep4 bf16
                with tc.tile_pool(name="l1ps", bufs=2, space="PSUM") as l1ps:
                    for c in range(NCH):
                        pxp = l1ps.tile([128, NX], F32, tag="l1")
                        for k in range(KT):
                            nc.tensor.matmul(
                                pxp[:, :],
                                w1slab(w1x_g, c, k),
                                xt_sb[:, ts(k, NX)],
                                start=(k == 0),
                                stop=(k == KT - 1),
                            )
                        with tc.high_priority():
                            nc.scalar.activation(
                                out=negs_sb[:, ts(c, 128)],
                                in_=pxp[:, :],
                                func=mybir.ActivationFunctionType.Identity,
                                bias=b1_sb[:, NCH + c : NCH + c + 1],
                                scale=-1.0,
                            )
                            nc.scalar.activation(
                                out=s_sb[:, ts(c, 128)],
                                in_=pxp[:, :],
                                func=mybir.ActivationFunctionType.Identity,
                                bias=b1_sb[:, c : c + 1],
                                scale=1.0,
                            )
                        pyp = l1ps.tile([128, NY], F32, tag="l1")
                        for k in range(KT):
                            nc.tensor.matmul(
                                pyp[:, :],
                                w1slab(w1y_g, c, k),
                                yt_sb[:, ts(k, NY)],
                                start=(k == 0),
                                stop=(k == KT - 1),
                            )
                        with tc.high_priority():
                            nc.scalar.activation(
                                out=pyr_sb[:, ts(c, 512)].rearrange(
                                    "p (m j) -> p m j", j=4
                                ),
                                in_=pyp[:, :].unsqueeze(2).broadcast_to(
                                    (128, 128, 4)
                                ),
                                func=mybir.ActivationFunctionType.Copy,
                            )

                # banks 6,7 reuse layer-1's psum space (deps via allocator)
                mpsB_cm = tc.tile_pool(name="mpsB", bufs=1, space="PSUM")
                mpsB = mpsB_cm.__enter__()
                obanks[6] = mpsB.tile([128, 512], F32, name="ob6", tag="ob6")
                obanks[7] = mpsB.tile([128, 512], F32, name="ob7", tag="ob7")

                # ---- main loop, c-outer; slivers accumulate across chunks
                for c in range(NCH):
                    last = c == NCH - 1
                    pyr_c = pyr_sb[:, ts(c, 512)]
                    pyr3 = pyr_c.rearrange("p (m j) -> p m j", j=4)
                    tslice = {}  # nb -> (tile, column offset index)

                    def dve_max(t, w, in1, prio):
                        in0 = pyr3.unsqueeze(1).broadcast_to((128, w, 128, 4))
                        out_ap = t[:, :].rearrange(
                            "p (nbs m j) -> p nbs m j", nbs=w, m=128
                        )
                        if prio:
                            with tc.high_priority():
                                nc.vector.tensor_tensor(
                                    out=out_ap, in0=in0, in1=in1,
                                    op=mybir.AluOpType.max,
                                )
                        else:
                            nc.vector.tensor_tensor(
                                out=out_ap, in0=in0, in1=in1,
                                op=mybir.AluOpType.max,
                            )

                    def negs_in1(cols, w):
                        # cols: list-slice of negs columns [p, w, 4] -> bcast m
                        return cols.unsqueeze(2).broadcast_to((128, w, 128, 4))

                    if last:
                        # bank-grouped w4 ops so the final bank-major
                        # matmul+evac+DMA pipeline starts per-bank
                        nrr = negs_sb[:, ts(c, 128)].rearrange(
                            "p (nb j) -> p nb j", j=4
                        )
                        for bk in range(8):
                            nbs_list = [nb for nb in range(NBLK) if bankmap(nb)[0] == bk]
                            lo = nbs_list[0]
                            step = nbs_list[1] - nbs_list[0]
                            t = tp.tile([128, 4 * 512], BF16, name=f"tb{bk}", tag="t4")
                            in1 = negs_in1(nrr[:, lo : lo + 3 * step + 1 : step, :], 4)
                            dve_max(t, 4, in1, False)
                            for i, nb in enumerate(nbs_list):
                                tslice[nb] = (t, i)
                    elif c == 0:
                        for gi in range(3):
                            lo = gi * 8
                            t = tp.tile([128, 8 * 512], BF16, name=f"t{c}_{gi}", tag="t")
                            in1 = negs_in1(
                                negs_sb[:, c * 128 + lo * 4 : c * 128 + lo * 4 + 32]
                                .rearrange("p (nbs j) -> p nbs j", j=4), 8
                            )
                            dve_max(t, 8, in1, True)
                            for nbs in range(8):
                                tslice[lo + nbs] = (t, nbs)
                        tl = tp.tile([128, 8 * 512], BF16, name="tl0", tag="tl", bufs=2)
                        in1 = negs_in1(
                            negs_sb[:, c * 128 + 96 : c * 128 + 128]
                            .rearrange("p (nbs j) -> p nbs j", j=4), 8
                        )
                        dve_max(tl, 8, in1, True)
                        for nbs in range(8):
                            tslice[24 + nbs] = (tl, nbs)
                    else:
                        for gi in range(3):
                            lo = gi * 8
                            t = tp.tile([128, 8 * 512], BF16, name=f"t{c}_{gi}", tag="t")
                            in1 = negs_in1(
                                negs_sb[:, c * 128 + lo * 4 : c * 128 + lo * 4 + 32]
                                .rearrange("p (nbs j) -> p nbs j", j=4), 8
                            )
                            dve_max(t, 8, in1, False)
                            for nbs in range(8):
                                tslice[lo + nbs] = (t, nbs)
                        t4 = tp.tile([128, 4 * 512], BF16, name=f"t4_{c}", tag="t4")
                        in1 = negs_in1(
                            negs_sb[:, c * 128 + 96 : c * 128 + 112]
                            .rearrange("p (nbs j) -> p nbs j", j=4), 4
                        )
                        dve_max(t4, 4, in1, False)
                        for nbs in range(4):
                            tslice[24 + nbs] = (t4, nbs)
                        # ACT share (relu-form rows, bank 6)
                        for nb in ACT_NBS:
                            ta = tp.tile(
                                [128, 512], BF16, name=f"ta{c}_{nb}", tag="ta", bufs=4
                            )
                            for j in range(4):
                                n = nb * 4 + j
                                nc.scalar.activation(
                                    out=ta[:, :].rearrange("p (m j) -> p m j", j=4)[
                                        :, :, j
                                    ],
                                    in_=pyr3[:, :, j],
                                    func=mybir.ActivationFunctionType.Relu,
                                    bias=s_sb[:, c * 128 + n : c * 128 + n + 1],
                                    scale=1.0,
                                )
                            tslice[nb] = (ta, 0)

                    if not last:
                        for nb in range(NBLK):
                            bk, jc = bankmap(nb)
                            t, nbs = tslice[nb]
                            nc.tensor.matmul(
                                obanks[bk][32 * jc : 32 * jc + 32, :],
                                w2_sb[:, ts(c, 32)],
                                t[:, ts(nbs, 512)],
                                start=(c == 0),
                                stop=False,
                                tile_position=(0, 32 * jc),
                                skip_group_check=True,
                            )
                    else:
                        # bank-major: 4 slivers -> ScalarE evac -> out-DMA
                        for bk in range(8):
                            for nb in range(NBLK):
                                b2_, jc = bankmap(nb)
                                if b2_ != bk:
                                    continue
                                t, nbs = tslice[nb]
                                nc.tensor.matmul(
                                    obanks[bk][32 * jc : 32 * jc + 32, :],
                                    w2_sb[:, ts(c, 32)],
                                    t[:, ts(nbs, 512)],
                                    start=False,
                                    stop=True,
                                    tile_position=(0, 32 * jc),
                                    skip_group_check=True,
                                )
                            nc.scalar.copy(
                                out=out_sc[:, ts(bk, 512)], in_=obanks[bk][:, :]
                            )
                            # raw layout: raw[nb*512 + m*4 + j] for the 4 nb
                            # of this bank (jc = 0..3 at partitions 0/32/64/96)
                            nbs_list = [nb for nb in range(NBLK) if bankmap(nb)[0] == bk]
                            lo = nbs_list[0]
                            step = nbs_list[1] - nbs_list[0]
                            dst = out[:, :].rearrange(
                                "o (nb q) -> (o nb) q", nb=NBLK
                            )[lo : lo + 3 * step + 1 : step, :]
                            src = out_sc[0:128:32, ts(bk, 512)]
                            (nc.sync, nc.scalar)[bk % 2].dma_start(out=dst, in_=src)
                mpsB_cm.__exit__(None, None, None)

    if do_compile:
        nc.compile()
    return nc


_NC_CACHE = None


def _get_nc():
    global _NC_CACHE
    if _NC_CACHE is None:
        _NC_CACHE = _build_nc()
    return _NC_CACHE


def prepare_in_maps(X, Y, W1, b1, W2):
    X = np.asarray(X, dtype=np.float32)
    Y = np.asarray(Y, dtype=np.float32)
    W1 = np.asarray(W1, dtype=np.float32)
    b1 = np.asarray(b1, dtype=np.float32)
    W2 = np.asarray(W2, dtype=np.float32)

    bf = ml_dtypes.bfloat16

    def w1_img(Wh):  # (HD, D) -> flat sbuf image (128, KT*HD)
        return np.ascontiguousarray(
            Wh.reshape(NCH, 128, KT, 128).transpose(3, 0, 2, 1).reshape(128, -1)
        ).astype(bf)

    def xy_img(Xb):  # (N, D) -> flat sbuf image (128, KT*N)
        return np.ascontiguousarray(
            Xb.T.reshape(KT, 128, -1).transpose(1, 0, 2).reshape(128, -1)
        ).astype(bf)

    w1xt = w1_img(W1[:, :D])
    w1yt = w1_img(W1[:, D:])
    b1m = b1.reshape(NCH, 128).T                      # (128, NCH)
    b1cm = np.ascontiguousarray(np.hstack([b1m, -b1m]))  # [+b1 | -b1] f32
    w2cm = np.ascontiguousarray(
        np.repeat(W2.reshape(NCH, 128).T[:, :, None], 32, axis=2).reshape(128, -1)
    ).astype(bf)

    in_maps = []
    for b in range(B):
        in_maps.append(
            {
                "xt": xy_img(X[b]),
                "yt": xy_img(Y[b]),
                "w1xt": w1xt,
                "w1yt": w1yt,
                "b1c": b1cm,
                "w2c": w2cm,
            }
        )
    return in_maps


def postprocess(raw_outs, X, W1, b1, W2, b2):
    """raw[nb*512 + m*4 + j] = device sum for out row 4nb+j, col m.
    Add the per-(row, chunk-set) max-form correction gamma, then b2."""
    X = np.asarray(X, dtype=np.float32)
    W1 = np.asarray(W1, dtype=np.float32)
    b1 = np.asarray(b1, dtype=np.float32)
    W2 = np.asarray(W2, dtype=np.float32)
    b2 = np.asarray(b2, dtype=np.float32)

    # per-chunk rank-1 pieces: gam_c[b, n] = X[b, n]·v_c + g_c
    Vc = np.stack([
        W2[0, c * 128 : (c + 1) * 128] @ W1[c * 128 : (c + 1) * 128, :D]
        for c in range(NCH)
    ])  # (NCH, D)
    gc = np.array([
        W2[0, c * 128 : (c + 1) * 128] @ b1[c * 128 : (c + 1) * 128]
        for c in range(NCH)
    ])
    maxform = np.ones((NBLK, NCH), dtype=np.float32)
    for nb in ACT_NBS:
        for c in range(NCH):
            if c not in ALLDVE_CHUNKS:
                maxform[nb, c] = 0.0

    out = np.empty((B, NX, NY), dtype=np.float32)
    for b in range(B):
        r = raw_outs[b].reshape(NBLK, 128, 4)     # (nb, m, j)
        o = r.transpose(0, 2, 1).reshape(NX, NY)  # (4nb+j, m)
        A = X[b] @ Vc.T + gc                      # (NX, NCH)
        gamma = (A.reshape(NBLK, 4, NCH) * maxform[:, None, :]).sum(-1)
        out[b] = o + gamma.reshape(NX)[:, None] + b2[0]
    return out


def kernel(X, Y, W1, b1, W2, b2):
    in_maps = prepare_in_maps(X, Y, W1, b1, W2)
    nc = _get_nc()
    res = run_bass_kernel_spmd(nc, in_maps, core_ids=list(range(NCORES)))
    raw = [res.results[b]["out"].reshape(-1) for b in range(B)]
    return postprocess(raw, X, W1, b1, W2, b2)


if __name__ == "__main__":
    rng = np.random.default_rng(0)
    ins = {
        "X": rng.standard_normal((B, NX, D), dtype=np.float32),
        "Y": rng.standard_normal((B, NY, D), dtype=np.float32),
        "W1": rng.standard_normal((HD, 2 * D), dtype=np.float32) * (2 * D) ** -0.5,
        "b1": rng.standard_normal((HD,), dtype=np.float32) * (2 * D) ** -0.5,
        "W2": rng.standard_normal((1, HD), dtype=np.float32) * HD**-0.5,
        "b2": rng.standard_normal((1,), dtype=np.float32) * HD**-0.5,
    }
    o = kernel(**ins)
    print("kernel out:", o.shape, o.dtype, float(np.abs(o).max()))
